# revision 1
# baseline (speedup 1.0000x reference)
"""Trainium2 Bass kernel for a 4-layer GINE graph encoder (GNN message passing).

Strategy (8 NeuronCores, SPMD):
  - Nodes sharded: core k owns rows [k*6250, (k+1)*6250), padded to 6272 (=49*128).
  - Edges partitioned by dst owner, sorted by dst, grouped into 128-dst
    segments; segment-sum is done on the tensor engine with host-built
    one-hot matrices (no scatter).
  - h[src] gathered per 128 edges from a replicated bf16 node table in DRAM
    (indirect DMA); table rebuilt each layer with an AllGather collective.
  - Bond encoder collapsed to a 512-row table (8^3 feature combos).
  - Atom embedding sums via one-hot matmuls.
  - GEMMs in bf16 (fp32 PSUM accumulation); LayerNorm in fp32 row-major.
"""
import numpy as np
import ml_dtypes
from contextlib import ExitStack

import concourse.bass as bass
import concourse.tile as tile
from concourse import bacc, mybir
from concourse.bass_utils import run_bass_kernel_spmd
from concourse.masks import make_identity

BF16 = mybir.dt.bfloat16
F32 = mybir.dt.float32
I32 = mybir.dt.int32
AF = mybir.ActivationFunctionType
ALU = mybir.AluOpType
bf = ml_dtypes.bfloat16

NCORES = 8
N, E, D, L = 50000, 160000, 512, 4
NPC = N // NCORES          # 6250 real nodes per core
NT = 49                    # node tiles per core
NPAD = NT * 128            # 6272 padded nodes per core
NTOT = NCORES * NPAD       # 50176
NSEG = NT                  # 49 segments of 128 dst slots per core
LN_EPS = 1e-5

_cache = {}


def _host_prep(x, edge_attr, edge_index):
    """Build per-core index/one-hot arrays."""
    x = np.asarray(x)
    ea = np.asarray(edge_attr)
    ei = np.asarray(edge_index)
    src, dst = ei[0].astype(np.int64), ei[1].astype(np.int64)
    combo_all = (ea[:, 0] * 64 + ea[:, 1] * 8 + ea[:, 2]).astype(np.int32)
    srcg = ((src // NPC) * NPAD + src % NPC).astype(np.int32)

    dstc = dst // NPC
    dstl = dst % NPC
    seg = dstl // 128
    m = (dstl % 128).astype(np.int64)

    gid = dstc * NSEG + seg
    cnt = np.bincount(gid, minlength=NCORES * NSEG).reshape(NCORES, NSEG)
    T = np.maximum(1, (cnt.max(0) + 127) // 128).astype(np.int64)
    ts0 = np.concatenate([[0], np.cumsum(T)])
    TS_TOT = int(ts0[-1])

    order = np.argsort(gid, kind="stable")
    gs = gid[order]
    starts = np.searchsorted(gs, np.arange(NCORES * NSEG))
    rank = np.arange(E) - starts[gs]
    col = ts0[seg[order]] + rank // 128
    row = rank % 128

    src_arr = np.zeros((NCORES, 128, TS_TOT), np.int32)
    cmb_arr = np.zeros((NCORES, 128, TS_TOT), np.int32)
    ohe = np.zeros((NCORES, 128, TS_TOT, 128), bf)
    ce = dstc[order]
    src_arr[ce, row, col] = srcg[order]
    cmb_arr[ce, row, col] = combo_all[order]
    ohe[ce, row, col, m[order]] = 1

    xp = np.zeros((NCORES, NPAD, 9), np.int64)
    xp[:, :NPC] = x.reshape(NCORES, NPC, 9)
    oha = np.zeros((NCORES, 128, NT, 9, 128), bf)
    kk, nn, ff = np.meshgrid(np.arange(NCORES), np.arange(NPAD), np.arange(9),
                             indexing="ij")
    oha[kk.ravel(), xp.ravel(), (nn // 128).ravel(), ff.ravel(),
        (nn % 128).ravel()] = 1

    ohb = np.zeros((24, 512), bf)
    c = np.arange(512)
    ohb[(c // 64), c] = 1
    ohb[8 + (c // 8) % 8, c] = 1
    ohb[16 + c % 8, c] = 1

    return dict(T=T, ts0=ts0, TS_TOT=TS_TOT, TMAX=int(T.max()),
                src_arr=src_arr, cmb_arr=cmb_arr, ohe=ohe, oha=oha, ohb=ohb)


def _w_sb_layout(w):
    return np.ascontiguousarray(
        np.asarray(w, np.float32).reshape(4, 128, 512).transpose(1, 0, 2)
    ).astype(bf)


def _b_layout(b):
    return np.ascontiguousarray(
        np.asarray(b, np.float32).reshape(4, 128).T).astype(np.float32)


def _repl(v):
    return np.ascontiguousarray(
        np.broadcast_to(np.asarray(v, np.float32), (128, 512)))


GROUPS = [list(range(g * 4, min(g * 4 + 4, NT))) for g in range(13)]


def build_program(T, ts0, TS_TOT, TMAX):
    nc = bacc.Bacc("TRN2", target_bir_lowering=False, debug=False,
                   num_devices=NCORES)

    def din(name, shape, dt):
        return nc.dram_tensor(name, shape, dt, kind="ExternalInput")

    srcidx = din("srcidx", [128, TS_TOT], I32)
    combo = din("combo", [128, TS_TOT], I32)
    ohe = din("ohe", [128, TS_TOT, 128], BF16)
    oha = din("oha", [128, NT, 9, 128], BF16)
    ohb = din("ohb", [24, 512], BF16)
    atom_emb = din("atom_emb", [128, 9, 512], BF16)
    bond_emb = din("bond_emb", [24, 512], BF16)
    aw1 = din("aw1", [128, 4, 512], BF16)
    aw2 = din("aw2", [128, 4, 512], BF16)
    bw1 = din("bw1", [128, 4, 512], BF16)
    bw2 = din("bw2", [128, 4, 512], BF16)
    cw1 = din("cw1", [L, 128, 4, 512], BF16)
    cw2 = din("cw2", [L, 128, 4, 512], BF16)
    ab1 = din("ab1", [128, 4], F32)
    ab2 = din("ab2", [128, 4], F32)
    bb1 = din("bb1", [128, 4], F32)
    bb2 = din("bb2", [128, 4], F32)
    cb1 = din("cb1", [L, 128, 4], F32)
    cb2 = din("cb2", [L, 128, 4], F32)
    aln = din("aln", [2, 128, 512], F32)
    bln = din("bln", [2, 128, 512], F32)
    cln = din("cln", [L, 2, 128, 512], F32)

    out_h = nc.dram_tensor("out_h", [NPAD, 512], F32, kind="ExternalOutput")

    h_own = [nc.dram_tensor(f"h_own{i}", [NPAD, 512], F32) for i in range(2)]
    shard = [nc.dram_tensor(f"shard{i}", [NPAD, 512], BF16) for i in range(2)]
    h_tab = [nc.dram_tensor(f"h_tab{i}", [NTOT, 512], BF16,
                            addr_space="Shared") for i in range(2)]
    e_table = nc.dram_tensor("e_table", [512, 512], BF16)
    e_edges = nc.dram_tensor("e_edges", [128, TS_TOT, 512], BF16)

    RG = [list(range(NCORES))]

    with tile.TileContext(nc) as tc:
        with ExitStack() as ctx:
            const = ctx.enter_context(tc.tile_pool(name="const", bufs=1))
            sb = ctx.enter_context(tc.tile_pool(name="sb", bufs=2))
            p_acc = ctx.enter_context(
                tc.tile_pool(name="p_acc", bufs=2, space="PSUM"))
            p_zt = ctx.enter_context(
                tc.tile_pool(name="p_zt", bufs=2, space="PSUM"))
            p_mm = ctx.enter_context(
                tc.tile_pool(name="p_mm", bufs=2, space="PSUM"))
            p_z2 = ctx.enter_context(
                tc.tile_pool(name="p_z2", bufs=2, space="PSUM"))

            ident = const.tile([128, 128], BF16)
            make_identity(nc, ident[:])
            eps_t = const.tile([128, 1], F32)
            nc.vector.memset(eps_t[:], LN_EPS)

            _cc = [0]

            def load_const(ap, shape, dt):
                _cc[0] += 1
                t = const.tile(shape, dt, tag=f"const{_cc[0]}",
                               name=f"const{_cc[0]}")
                nc.sync.dma_start(t[:], ap)
                return t

            atom_emb_s = load_const(atom_emb[:], [128, 9, 512], BF16)
            bond_emb_s = load_const(bond_emb[:], [24, 512], BF16)
            ohb_s = load_const(ohb[:], [24, 512], BF16)
            aw1_s = load_const(aw1[:], [128, 4, 512], BF16)
            aw2_s = load_const(aw2[:], [128, 4, 512], BF16)
            bw1_s = load_const(bw1[:], [128, 4, 512], BF16)
            bw2_s = load_const(bw2[:], [128, 4, 512], BF16)
            cw1_s = [load_const(cw1[l], [128, 4, 512], BF16) for l in range(L)]
            cw2_s = [load_const(cw2[l], [128, 4, 512], BF16) for l in range(L)]
            ab1_s = load_const(ab1[:], [128, 4], F32)
            ab2_s = load_const(ab2[:], [128, 4], F32)
            bb1_s = load_const(bb1[:], [128, 4], F32)
            bb2_s = load_const(bb2[:], [128, 4], F32)
            cb1_s = [load_const(cb1[l], [128, 4], F32) for l in range(L)]
            cb2_s = [load_const(cb2[l], [128, 4], F32) for l in range(L)]
            aln_g_s = load_const(aln[0], [128, 512], F32)
            aln_b_s = load_const(aln[1], [128, 512], F32)
            bln_g_s = load_const(bln[0], [128, 512], F32)
            bln_b_s = load_const(bln[1], [128, 512], F32)
            cln_g_s = [load_const(cln[l, 0], [128, 512], F32) for l in range(L)]
            cln_b_s = [load_const(cln[l, 1], [128, 512], F32) for l in range(L)]

            def mlp_block(rows, W, w1_s, b1_s, w2_s, b2_s, act1, evac):
                """rows: list of [128,512] bf16 SBUF tiles. evac(i, z2p) is
                called per output row-tile with the row-major bf16 PSUM tile."""
                nt = W // 128
                zT = sb.tile([128, 4, W], BF16, tag=f"zT{W}", bufs=2)
                for d in range(4):
                    ztp = p_zt.tile([128, W], BF16, tag="ztp")
                    for s in range(nt):
                        nc.tensor.transpose(ztp[:, s * 128:(s + 1) * 128],
                                            rows[s][:, d * 128:(d + 1) * 128],
                                            ident[:])
                    nc.scalar.activation(zT[:, d, :], ztp[:], AF.Copy)
                a1 = sb.tile([128, 4, W], BF16, tag=f"a1{W}", bufs=2)
                for mc in range(4):
                    mm = p_mm.tile([128, W], F32, tag="mm")
                    for kc in range(4):
                        nc.tensor.matmul(mm[:],
                                         w1_s[:, kc, mc * 128:(mc + 1) * 128],
                                         zT[:, kc, :],
                                         start=(kc == 0), stop=(kc == 3))
                    nc.scalar.activation(a1[:, mc, :], mm[:], act1,
                                         bias=b1_s[:, mc:mc + 1])
                z2T = sb.tile([128, 4, W], BF16, tag=f"z2T{W}", bufs=2)
                for mc in range(4):
                    mm = p_mm.tile([128, W], F32, tag="mm")
                    for kc in range(4):
                        nc.tensor.matmul(mm[:],
                                         w2_s[:, kc, mc * 128:(mc + 1) * 128],
                                         a1[:, kc, :],
                                         start=(kc == 0), stop=(kc == 3))
                    nc.scalar.activation(z2T[:, mc, :], mm[:], AF.Identity,
                                         bias=b2_s[:, mc:mc + 1])
                for s in range(nt):
                    z2p = p_z2.tile([128, 512], BF16, tag="z2p")
                    for d in range(4):
                        nc.tensor.transpose(z2p[:, d * 128:(d + 1) * 128],
                                            z2T[:, d, s * 128:(s + 1) * 128],
                                            ident[:])
                    evac(s, z2p)

            def ln_stats(rsum, ssq, G):
                mean = sb.tile([128, 4], F32, tag="mean")
                nc.vector.tensor_scalar_mul(mean[:, :G], rsum[:, :G], 1.0 / 512)
                m2 = sb.tile([128, 4], F32, tag="m2")
                nc.vector.tensor_mul(m2[:, :G], mean[:, :G], mean[:, :G])
                var = sb.tile([128, 4], F32, tag="var")
                nc.vector.scalar_tensor_tensor(var[:, :G], ssq[:, :G],
                                               1.0 / 512, m2[:, :G],
                                               op0=ALU.mult, op1=ALU.subtract)
                std = sb.tile([128, 4], F32, tag="std")
                nc.scalar.activation(std[:, :G], var[:, :G], AF.Sqrt,
                                     bias=eps_t[:])
                rstd = sb.tile([128, 4], F32, tag="rstd")
                nc.vector.reciprocal(rstd[:, :G], std[:, :G])
                nmrs = sb.tile([128, 4], F32, tag="nmrs")
                nc.vector.scalar_tensor_tensor(nmrs[:, :G], mean[:, :G], -1.0,
                                               rstd[:, :G],
                                               op0=ALU.mult, op1=ALU.mult)
                return rstd, nmrs

            def ln_apply_b16(rows_list, rsum, ssq, G, g_s, b_s, outs):
                rstd, nmrs = ln_stats(rsum, ssq, G)
                for s, rows in enumerate(rows_list):
                    xn = sb.tile([128, 512], F32, tag="xn")
                    nc.scalar.activation(xn[:], rows[:], AF.Identity,
                                         scale=rstd[:, s:s + 1],
                                         bias=nmrs[:, s:s + 1])
                    y = sb.tile([128, 512], F32, tag="y")
                    nc.vector.tensor_mul(y[:], xn[:], g_s[:])
                    nc.vector.tensor_add(outs[s][:], y[:], b_s[:])

            # ================= PHASE 1: bond table =================
            bond_rows = []
            rsum_b = sb.tile([128, 4], F32, tag="rsum")
            ssq_b = sb.tile([128, 4], F32, tag="ssq")
            for t in range(4):
                acc = p_acc.tile([128, 512], F32, tag="acc")
                nc.tensor.matmul(acc[:], ohb_s[:, t * 128:(t + 1) * 128],
                                 bond_emb_s[:], start=True, stop=True)
                rows = sb.tile([128, 512], F32, tag="embrows", bufs=6)
                nc.scalar.activation(rows[:], acc[:], AF.Identity,
                                     accum_out=rsum_b[:, t:t + 1])
                sq = sb.tile([128, 512], BF16, tag="sq")
                nc.scalar.activation(sq[:], rows[:], AF.Square,
                                     accum_out=ssq_b[:, t:t + 1])
                bond_rows.append(rows)

            bln_rows = [sb.tile([128, 512], BF16, tag="lnout", bufs=6,
                                 name=f"blnr{i}") for i in range(4)]
            ln_apply_b16(bond_rows, rsum_b, ssq_b, 4, bln_g_s, bln_b_s,
                         bln_rows)

            def bond_evac(s, z2p):
                eout = sb.tile([128, 512], BF16, tag="eout")
                nc.scalar.activation(eout[:], z2p[:], AF.Copy)
                nc.sync.dma_start(e_table[s * 128:(s + 1) * 128, :], eout[:])

            mlp_block(bln_rows, 512, bw1_s, bb1_s, bw2_s, bb2_s, AF.Gelu,
                      bond_evac)

            # ================= PHASE 2: materialize e_edges =================
            combo_s = const.tile([128, TS_TOT], I32)
            nc.sync.dma_start(combo_s[:], combo[:])
            srcidx_s = const.tile([128, TS_TOT], I32)
            nc.sync.dma_start(srcidx_s[:], srcidx[:])
            for ts in range(TS_TOT):
                et = sb.tile([128, 512], BF16, tag="emat", bufs=2)
                nc.gpsimd.indirect_dma_start(
                    out=et[:], out_offset=None, in_=e_table[:],
                    in_offset=bass.IndirectOffsetOnAxis(
                        ap=combo_s[:, ts:ts + 1], axis=0))
                nc.sync.dma_start(e_edges[:, ts, :], et[:])

            # ================= PHASE 3: atom encoder =================
            for grp in GROUPS:
                W = len(grp) * 128
                G = len(grp)
                rsum = sb.tile([128, 4], F32, tag="rsum")
                ssq = sb.tile([128, 4], F32, tag="ssq")
                rows_f = []
                for i, t in enumerate(grp):
                    oh = sb.tile([128, 9, 128], BF16, tag="oha", bufs=2)
                    nc.sync.dma_start(oh[:], oha[:, t, :, :])
                    acc = p_acc.tile([128, 512], F32, tag="acc")
                    for f in range(9):
                        nc.tensor.matmul(acc[:], oh[:, f, :],
                                         atom_emb_s[:, f, :],
                                         start=(f == 0), stop=(f == 8))
                    rows = sb.tile([128, 512], F32, tag="embrows", bufs=6)
                    nc.scalar.activation(rows[:], acc[:], AF.Identity,
                                         accum_out=rsum[:, i:i + 1])
                    sq = sb.tile([128, 512], BF16, tag="sq")
                    nc.scalar.activation(sq[:], rows[:], AF.Square,
                                         accum_out=ssq[:, i:i + 1])
                    rows_f.append(rows)
                lnr = [sb.tile([128, 512], BF16, tag="lnout", bufs=6,
                               name=f"lnr{i}") for i in range(G)]
                ln_apply_b16(rows_f, rsum, ssq, G, aln_g_s, aln_b_s, lnr)

                def atom_evac(i, z2p, grp=grp):
                    t = grp[i]
                    hf = sb.tile([128, 512], F32, tag="hf")
                    nc.scalar.activation(hf[:], z2p[:], AF.Copy)
                    hb = sb.tile([128, 512], BF16, tag="hb")
                    nc.vector.tensor_copy(hb[:], z2p[:])
                    nc.sync.dma_start(h_own[0][t * 128:(t + 1) * 128, :], hf[:])
                    nc.sync.dma_start(shard[0][t * 128:(t + 1) * 128, :], hb[:])

                mlp_block(lnr, W, aw1_s, ab1_s, aw2_s, ab2_s, AF.Gelu,
                          atom_evac)
            nc.gpsimd.collective_compute(
                "AllGather", ALU.bypass, replica_groups=RG,
                ins=[shard[0][:]], outs=[h_tab[0][:]])

            # ================= PHASE 4: conv layers =================
            for l in range(L):
                tab = h_tab[l % 2]
                own_src = h_own[l % 2]
                own_dst = h_own[(l + 1) % 2]
                shd = shard[(l + 1) % 2]
                g_s, b_s = cln_g_s[l], cln_b_s[l]
                for grp in GROUPS:
                    W = len(grp) * 128
                    G = len(grp)
                    z_rows = []
                    hin_tiles = []
                    for s in grp:
                        t0, t1 = int(ts0[s]), int(ts0[s + 1])
                        Ts = t1 - t0
                        hs = sb.tile([128, TMAX, 512], BF16, tag="hs", bufs=2)
                        for t in range(Ts):
                            nc.gpsimd.indirect_dma_start(
                                out=hs[:, t, :], out_offset=None, in_=tab[:],
                                in_offset=bass.IndirectOffsetOnAxis(
                                    ap=srcidx_s[:, t0 + t:t0 + t + 1], axis=0))
                        ee = sb.tile([128, TMAX, 512], BF16, tag="ee", bufs=2)
                        nc.sync.dma_start(ee[:, :Ts, :], e_edges[:, t0:t1, :])
                        msg = hs
                        nc.any.tensor_add(msg[:, :Ts, :], hs[:, :Ts, :],
                                          ee[:, :Ts, :])
                        nc.any.tensor_scalar_max(msg[:, :Ts, :],
                                                 msg[:, :Ts, :], 0.0)
                        oh = sb.tile([128, TMAX, 128], BF16, tag="ohe", bufs=3)
                        nc.sync.dma_start(oh[:, :Ts, :], ohe[:, t0:t1, :])
                        agg = p_acc.tile([128, 512], F32, tag="acc")
                        for t in range(Ts):
                            nc.tensor.matmul(agg[:], oh[:, t, :], msg[:, t, :],
                                             start=(t == 0), stop=(t == Ts - 1))
                        hin = sb.tile([128, 512], F32, tag="hin", bufs=5)
                        nc.sync.dma_start(
                            hin[:], own_src[s * 128:(s + 1) * 128, :])
                        z = sb.tile([128, 512], BF16, tag="z", bufs=6)
                        nc.vector.tensor_add(z[:], agg[:], hin[:])
                        z_rows.append(z)
                        hin_tiles.append(hin)

                    rsum = sb.tile([128, 4], F32, tag="rsum")
                    ssq = sb.tile([128, 4], F32, tag="ssq")
                    r_tiles = []

                    def conv_evac(i, z2p, hin_tiles=hin_tiles, rsum=rsum,
                                  ssq=ssq, r_tiles=r_tiles):
                        g2 = sb.tile([128, 512], F32, tag="g2", bufs=2)
                        nc.scalar.activation(g2[:], z2p[:], AF.Gelu)
                        r = sb.tile([128, 512], F32, tag="r", bufs=5)
                        nc.vector.scalar_tensor_tensor(
                            r[:], g2[:], 0.0, hin_tiles[i][:],
                            op0=ALU.bypass, op1=ALU.add,
                            accum_out=rsum[:, i:i + 1])
                        sq = sb.tile([128, 512], BF16, tag="sq")
                        nc.scalar.activation(sq[:], r[:], AF.Square,
                                             accum_out=ssq[:, i:i + 1])
                        r_tiles.append(r)

                    mlp_block(z_rows, W, cw1_s[l], cb1_s[l], cw2_s[l],
                              cb2_s[l], AF.Relu, conv_evac)

                    rstd, nmrs = ln_stats(rsum, ssq, G)
                    for i, s in enumerate(grp):
                        xn = sb.tile([128, 512], F32, tag="xn")
                        nc.scalar.activation(xn[:], r_tiles[i][:], AF.Identity,
                                             scale=rstd[:, i:i + 1],
                                             bias=nmrs[:, i:i + 1])
                        y = sb.tile([128, 512], F32, tag="y")
                        nc.vector.tensor_mul(y[:], xn[:], g_s[:])
                        hf = sb.tile([128, 512], F32, tag="hf")
                        nc.vector.tensor_add(hf[:], y[:], b_s[:])
                        rs = slice(s * 128, (s + 1) * 128)
                        if l == L - 1:
                            nc.sync.dma_start(out_h[rs, :], hf[:])
                        else:
                            nc.sync.dma_start(own_dst[rs, :], hf[:])
                            hb = sb.tile([128, 512], BF16, tag="hb")
                            nc.vector.tensor_copy(hb[:], hf[:])
                            nc.sync.dma_start(shd[rs, :], hb[:])
                if l < L - 1:
                    nc.gpsimd.collective_compute(
                        "AllGather", ALU.bypass, replica_groups=RG,
                        ins=[shd[:]], outs=[h_tab[(l + 1) % 2][:]])

    nc.compile()
    return nc


def kernel(x, edge_attr, edge_index,
           atom_emb, atom_ln_g, atom_ln_b, atom_w1, atom_b1, atom_w2, atom_b2,
           bond_emb, bond_ln_g, bond_ln_b, bond_w1, bond_b1, bond_w2, bond_b2,
           conv_w1, conv_b1, conv_w2, conv_b2, ln_g, ln_b):
    prep = _host_prep(x, edge_attr, edge_index)
    T, ts0, TS_TOT = prep["T"], prep["ts0"], prep["TS_TOT"]

    key = (TS_TOT, tuple(T.tolist()))
    if key not in _cache:
        _cache[key] = build_program(T, ts0, TS_TOT, prep["TMAX"])
    nc = _cache[key]

    shared = dict(
        ohb=prep["ohb"],
        atom_emb=np.ascontiguousarray(
            np.asarray(atom_emb, np.float32).transpose(1, 0, 2)).astype(bf),
        bond_emb=np.asarray(bond_emb, np.float32).reshape(24, 512).astype(bf),
        aw1=_w_sb_layout(atom_w1), aw2=_w_sb_layout(atom_w2),
        bw1=_w_sb_layout(bond_w1), bw2=_w_sb_layout(bond_w2),
        cw1=np.stack([_w_sb_layout(conv_w1[l]) for l in range(L)]),
        cw2=np.stack([_w_sb_layout(conv_w2[l]) for l in range(L)]),
        ab1=_b_layout(atom_b1), ab2=_b_layout(atom_b2),
        bb1=_b_layout(bond_b1), bb2=_b_layout(bond_b2),
        cb1=np.stack([_b_layout(conv_b1[l]) for l in range(L)]),
        cb2=np.stack([_b_layout(conv_b2[l]) for l in range(L)]),
        aln=np.stack([_repl(atom_ln_g), _repl(atom_ln_b)]),
        bln=np.stack([_repl(bond_ln_g), _repl(bond_ln_b)]),
        cln=np.stack([np.stack([_repl(ln_g[l]), _repl(ln_b[l])])
                      for l in range(L)]),
    )
    in_maps = []
    for k in range(NCORES):
        m = dict(shared)
        m["srcidx"] = prep["src_arr"][k]
        m["combo"] = prep["cmb_arr"][k]
        m["ohe"] = prep["ohe"][k]
        m["oha"] = prep["oha"][k]
        in_maps.append(m)

    res = run_bass_kernel_spmd(nc, in_maps, list(range(NCORES)))
    kernel._last_results = res
    out = np.empty((N, D), np.float32)
    for k in range(NCORES):
        out[k * NPC:(k + 1) * NPC] = np.asarray(
            res.results[k]["out_h"], np.float32)[:NPC]
    return out



# revision 10
# speedup vs baseline: 6.2999x; 6.2999x over previous
"""Trainium2 Bass kernel for a 4-layer GINE graph encoder (GNN message passing).

Strategy (8 NeuronCores, SPMD):
  - Nodes sharded: core k owns rows [k*6250, (k+1)*6250), padded to 6272 (=49*128).
  - Edges partitioned by dst owner, sorted by (dst-segment, src-half); per-edge
    h[src] rows fetched with the custom DMAGatherAnt instruction (int16 indices,
    so the 50176-row replicated node table is split at row 31360 into lo/hi
    views and each 128-edge tile draws from a single half).
  - e_edges (bond features per edge) materialized once, then fused into the
    gathered h rows with an SWDGE accumulate-DMA (CCE inline add).
  - segment-sum via one-hot matmuls; one-hot matrices generated on-chip with
    iota==m compares on the vector engine (no DRAM one-hot traffic).
  - MLPs: GEMM1 keeps weights stationary (input transposed on the PE);
    GEMM2 uses the activation as the stationary operand so the output comes
    out row-major - no output transposes.
  - Residual/LN in row-major fp32; h table rebuilt per layer via AllGather.
"""
import numpy as np
import ml_dtypes
from contextlib import ExitStack

import concourse.bass as bass
import concourse.tile as tile
from concourse import bacc, mybir
from concourse.bass_utils import run_bass_kernel_spmd
from concourse.masks import make_identity

BF16 = mybir.dt.bfloat16
F32 = mybir.dt.float32
I16 = mybir.dt.int16
AF = mybir.ActivationFunctionType
ALU = mybir.AluOpType
bf = ml_dtypes.bfloat16

NCORES = 8
N, E, D, L = 50000, 160000, 512, 4
NPC = N // NCORES          # 6250 real nodes per core
NT = 49                    # node tiles per core
NPAD = NT * 128            # 6272 padded nodes per core
NTOT = NCORES * NPAD       # 50176
NSEG = NT                  # 49 segments of 128 dst slots per core
LO_CORES = 5
LO_N = LO_CORES * NPAD     # 31360 (< 32768 so int16 indices work)
HI_N = NTOT - LO_N         # 18816
LN_EPS = 1e-5

GROUPS = [list(range(a, min(a + 4, NSEG))) for a in range(0, NSEG, 4)]

_cache = {}


def _wrap16(lin):
    """[n*128] int array -> [128, n*8] int16 wrapped-in-16-partitions layout,
    replicated across the 8 GPSIMD cores (partition groups of 16)."""
    a = lin.reshape(-1, 16).T.astype(np.int16)           # [16, cols]
    return np.ascontiguousarray(
        np.broadcast_to(a[None], (8, 16, a.shape[1])).reshape(128, -1))


def _host_prep(x, edge_attr, edge_index):
    x = np.asarray(x)
    ea = np.asarray(edge_attr)
    ei = np.asarray(edge_index)
    src, dst = ei[0].astype(np.int64), ei[1].astype(np.int64)
    combo = (ea[:, 0] * 64 + ea[:, 1] * 8 + ea[:, 2]).astype(np.int64)
    srcg = (src // NPC) * NPAD + src % NPC
    half = (srcg >= LO_N).astype(np.int64)               # 0 = lo, 1 = hi
    dstc = dst // NPC
    dstl = dst % NPC
    seg = dstl // 128
    m = dstl % 128

    gid = ((dstc * NSEG) + seg) * 2 + half
    cnt = np.bincount(gid, minlength=NCORES * NSEG * 2).reshape(
        NCORES, NSEG, 2)
    mx = cnt.max(0)
    Tl = (mx[:, 0] + 127) // 128
    Th = (mx[:, 1] + 127) // 128
    Tl[(Tl + Th) == 0] = 1

    # group-major tile streams
    gmeta = []       # per group: dict with static structure
    gts = 0          # full-stream tile counter
    lts = 0          # lo-stream tile counter
    hts = 0
    for grp in GROUPS:
        TLg = int(Tl[grp].sum())
        THg = int(Th[grp].sum())
        lo_off = {}
        hi_off = {}
        o = 0
        for s in grp:
            lo_off[s] = o
            o += int(Tl[s])
        o = 0
        for s in grp:
            hi_off[s] = o
            o += int(Th[s])
        gmeta.append(dict(grp=grp, gts0=gts, TLg=TLg, THg=THg,
                          lo_off=lo_off, hi_off=hi_off, lts0=lts, hts0=hts))
        gts += TLg + THg
        lts += TLg
        hts += THg
    TS = gts
    LO_T = lts
    HI_T = hts
    NTG_MAX = max(g["TLg"] + g["THg"] for g in gmeta)

    # per-edge slot assignment
    order = np.lexsort((np.arange(E), half, seg, dstc))
    gs = gid[order]
    starts = np.searchsorted(gs, np.arange(NCORES * NSEG * 2) * 1, side="left")
    # rank within (core, seg, half)
    first = np.zeros(E, np.int64)
    first = starts[gs]
    rank = np.arange(E) - first
    tw = rank // 128
    row = rank % 128

    seg_o = seg[order]
    half_o = half[order]
    core_o = dstc[order]
    # map seg -> group idx and offsets
    seg2g = np.empty(NSEG, np.int64)
    for gi, g in enumerate(gmeta):
        for s in g["grp"]:
            seg2g[s] = gi
    g_o = seg2g[seg_o]
    gts0_a = np.array([g["gts0"] for g in gmeta])
    TLg_a = np.array([g["TLg"] for g in gmeta])
    lts0_a = np.array([g["lts0"] for g in gmeta])
    hts0_a = np.array([g["hts0"] for g in gmeta])
    lo_off_a = np.zeros(NSEG, np.int64)
    hi_off_a = np.zeros(NSEG, np.int64)
    for gi, g in enumerate(gmeta):
        for s in g["grp"]:
            lo_off_a[s] = g["lo_off"][s]
            hi_off_a[s] = g["hi_off"][s]

    # full-stream tile index of each edge
    fs = np.where(
        half_o == 0,
        gts0_a[g_o] + lo_off_a[seg_o] + tw,
        gts0_a[g_o] + TLg_a[g_o] + hi_off_a[seg_o] + tw)
    # gather-stream tile index
    gsi = np.where(half_o == 0,
                   lts0_a[g_o] + lo_off_a[seg_o] + tw,
                   hts0_a[g_o] + hi_off_a[seg_o] + tw)

    srcg_o = srcg[order]
    cmb_o = combo[order]
    m_o = m[order]

    idx_lo = np.zeros((NCORES, LO_T * 128), np.int64)
    idx_hi = np.zeros((NCORES, HI_T * 128), np.int64)
    cmb_lin = np.zeros((NCORES, TS * 128), np.int64)
    m_lin = np.full((NCORES, TS * 128), -1, np.int64)

    lo_m = half_o == 0
    hi_m = ~lo_m
    idx_lo[core_o[lo_m], gsi[lo_m] * 128 + row[lo_m]] = srcg_o[lo_m]
    idx_hi[core_o[hi_m], gsi[hi_m] * 128 + row[hi_m]] = srcg_o[hi_m] - LO_N
    cmb_lin[core_o, fs * 128 + row] = cmb_o
    m_lin[core_o, fs * 128 + row] = m_o

    idx_lo_w = np.stack([_wrap16(idx_lo[k]) for k in range(NCORES)])
    idx_hi_w = np.stack([_wrap16(idx_hi[k]) for k in range(NCORES)])
    cmb_w = np.stack([_wrap16(cmb_lin[k]) for k in range(NCORES)])
    m_bf = np.ascontiguousarray(
        m_lin.reshape(NCORES, TS, 128).transpose(0, 2, 1)).astype(np.float32)

    # atom one-hot: [core, vocab-part, tile, feat, node-col]
    xp = np.zeros((NCORES, NPAD, 9), np.int64)
    xp[:, :NPC] = x.reshape(NCORES, NPC, 9)
    oha = np.zeros((NCORES, 128, NT, 9, 128), bf)
    kk, nn, ff = np.meshgrid(np.arange(NCORES), np.arange(NPAD), np.arange(9),
                             indexing="ij")
    oha[kk.ravel(), xp.ravel(), (nn // 128).ravel(), ff.ravel(),
        (nn % 128).ravel()] = 1

    iota = np.broadcast_to(np.arange(128, dtype=np.float32), (128, 128))
    iota = np.ascontiguousarray(iota).astype(bf)

    ohb = np.zeros((24, 512), bf)
    c = np.arange(512)
    ohb[(c // 64), c] = 1
    ohb[8 + (c // 8) % 8, c] = 1
    ohb[16 + c % 8, c] = 1

    key = (TS, tuple(Tl.tolist()), tuple(Th.tolist()))
    return dict(Tl=Tl, Th=Th, gmeta=gmeta, TS=TS, LO_T=LO_T, HI_T=HI_T,
                NTG_MAX=NTG_MAX, idx_lo=idx_lo_w, idx_hi=idx_hi_w,
                cmb=cmb_w, m_bf=m_bf, oha=oha, iota=iota, ohb=ohb, key=key)


def _w_sb_layout(w):
    """[512,512] -> [128, 4, 512] bf16: [p, c, o] = w[c*128+p, o].
    Serves as GEMM1 stationary (lhsT slices) AND GEMM2 moving operand."""
    return np.ascontiguousarray(
        np.asarray(w, np.float32).reshape(4, 128, 512).transpose(1, 0, 2)
    ).astype(bf)


def _b_layout(b):
    return np.ascontiguousarray(
        np.asarray(b, np.float32).reshape(4, 128).T).astype(np.float32)


def _repl_bf(v):
    return np.ascontiguousarray(
        np.broadcast_to(np.asarray(v, np.float32), (128, 512))).astype(bf)


def build_program(prep):
    Tl, Th, gmeta = prep["Tl"], prep["Th"], prep["gmeta"]
    TS, LO_T, HI_T = prep["TS"], prep["LO_T"], prep["HI_T"]
    NTG_MAX = prep["NTG_MAX"]

    nc = bacc.Bacc("TRN2", target_bir_lowering=False, debug=False,
                   num_devices=NCORES)

    def din(name, shape, dt):
        return nc.dram_tensor(name, shape, dt, kind="ExternalInput")

    idx_lo = din("idx_lo", [128, LO_T * 8], I16)
    idx_hi = din("idx_hi", [128, HI_T * 8], I16)
    cmb = din("cmb", [128, TS * 8], I16)
    m_bf = din("m_bf", [128, TS], F32)
    oha_d = din("oha", [128, NT, 9, 128], BF16)
    iota_in = din("iota", [128, 128], BF16)
    ohb = din("ohb", [24, 512], BF16)
    atom_emb = din("atom_emb", [128, 9, 512], BF16)
    bond_emb = din("bond_emb", [24, 512], BF16)
    aw1 = din("aw1", [128, 4, 512], BF16)
    aw2 = din("aw2", [128, 4, 512], BF16)
    bw1 = din("bw1", [128, 4, 512], BF16)
    bw2 = din("bw2", [128, 4, 512], BF16)
    cw1 = din("cw1", [L, 128, 4, 512], BF16)
    cw2 = din("cw2", [L, 128, 4, 512], BF16)
    ab1 = din("ab1", [128, 4], F32)
    bb1 = din("bb1", [128, 4], F32)
    cb1 = din("cb1", [L, 128, 4], F32)
    ab2r = din("ab2r", [128, 512], BF16)
    bb2r = din("bb2r", [128, 512], BF16)
    cb2r = din("cb2r", [L, 128, 512], BF16)
    aln = din("aln", [2, 128, 512], BF16)
    bln = din("bln", [2, 128, 512], BF16)
    cln = din("cln", [L, 2, 128, 512], BF16)

    out_h = nc.dram_tensor("out_h", [NPAD, 512], F32, kind="ExternalOutput")

    shard = [nc.dram_tensor(f"shard{i}", [NPAD, 512], BF16) for i in range(2)]
    h_tab = [nc.dram_tensor(f"h_tab{i}", [NTOT, 512], BF16,
                            addr_space="Shared") for i in range(2)]
    e_table = nc.dram_tensor("e_table", [512, 512], BF16)
    e_edges = nc.dram_tensor("e_edges", [128, TS, 512], BF16)

    RG = [list(range(NCORES))]

    with tile.TileContext(nc) as tc:
        with ExitStack() as ctx:
            const = ctx.enter_context(tc.tile_pool(name="const", bufs=1))
            cwp = ctx.enter_context(tc.tile_pool(name="cwp", bufs=2))
            sb = ctx.enter_context(tc.tile_pool(name="sb", bufs=2))
            p_acc = ctx.enter_context(
                tc.tile_pool(name="p_acc", bufs=2, space="PSUM"))
            p_zt = ctx.enter_context(
                tc.tile_pool(name="p_zt", bufs=2, space="PSUM"))
            p_mm = ctx.enter_context(
                tc.tile_pool(name="p_mm", bufs=2, space="PSUM"))
            p_z2 = ctx.enter_context(
                tc.tile_pool(name="p_z2", bufs=2, space="PSUM"))

            ident = const.tile([128, 128], BF16)
            make_identity(nc, ident[:])
            eps_t = const.tile([128, 1], F32)
            nc.vector.memset(eps_t[:], LN_EPS)

            _cc = [0]

            def load_const(ap, shape, dt):
                _cc[0] += 1
                t = const.tile(shape, dt, tag=f"const{_cc[0]}",
                               name=f"const{_cc[0]}")
                nc.sync.dma_start(t[:], ap)
                return t

            iota_s = load_const(iota_in[:], [128, 128], BF16)
            idx_lo_s = load_const(idx_lo[:], [128, LO_T * 8], I16)
            idx_hi_s = load_const(idx_hi[:], [128, HI_T * 8], I16)
            cmb_s = load_const(cmb[:], [128, TS * 8], I16)
            m_bf_s = load_const(m_bf[:], [128, TS], F32)
            atom_emb_s = load_const(atom_emb[:], [128, 9, 512], BF16)
            bond_emb_s = load_const(bond_emb[:], [24, 512], BF16)
            ohb_s = load_const(ohb[:], [24, 512], BF16)
            aw1_s = load_const(aw1[:], [128, 4, 512], BF16)
            aw2_s = load_const(aw2[:], [128, 4, 512], BF16)
            bw1_s = load_const(bw1[:], [128, 4, 512], BF16)
            bw2_s = load_const(bw2[:], [128, 4, 512], BF16)
            ab1_s = load_const(ab1[:], [128, 4], F32)
            bb1_s = load_const(bb1[:], [128, 4], F32)
            ab2r_s = load_const(ab2r[:], [128, 512], BF16)
            bb2r_s = load_const(bb2r[:], [128, 512], BF16)
            aln_g_s = load_const(aln[0], [128, 512], BF16)
            aln_b_s = load_const(aln[1], [128, 512], BF16)
            bln_g_s = load_const(bln[0], [128, 512], BF16)
            bln_b_s = load_const(bln[1], [128, 512], BF16)

            def ln_stats(rsum, ssq, G):
                mean = sb.tile([128, 4], F32, tag="mean")
                nc.vector.tensor_scalar_mul(mean[:, :G], rsum[:, :G], 1.0 / 512)
                m2 = sb.tile([128, 4], F32, tag="m2")
                nc.vector.tensor_mul(m2[:, :G], mean[:, :G], mean[:, :G])
                var = sb.tile([128, 4], F32, tag="var")
                nc.vector.scalar_tensor_tensor(var[:, :G], ssq[:, :G],
                                               1.0 / 512, m2[:, :G],
                                               op0=ALU.mult, op1=ALU.subtract)
                std = sb.tile([128, 4], F32, tag="std")
                nc.scalar.activation(std[:, :G], var[:, :G], AF.Sqrt,
                                     bias=eps_t[:])
                rstd = sb.tile([128, 4], F32, tag="rstd")
                nc.vector.reciprocal(rstd[:, :G], std[:, :G])
                nmrs = sb.tile([128, 4], F32, tag="nmrs")
                nc.vector.scalar_tensor_tensor(nmrs[:, :G], mean[:, :G], -1.0,
                                               rstd[:, :G],
                                               op0=ALU.mult, op1=ALU.mult)
                return rstd, nmrs

            def mlp(rows, G, w1_s, b1_s, w2_s, b2r_s, act1, evac):
                """rows: list of G row-major [128,512] bf16 tiles.
                GEMM1: w1 stationary, transposed rows moving -> hidden-major.
                GEMM2: a1 stationary, w2 moving -> row-major z2 in PSUM.
                evac(i, z2_psum) consumes each row-tile's output."""
                W = G * 128
                zT = sb.tile([128, 4, 512], BF16, tag="zT", bufs=2)
                for i in range(G):
                    ztp = p_zt.tile([128, 512], BF16, tag="ztp")
                    for d in range(4):
                        nc.tensor.transpose(ztp[:, d * 128:(d + 1) * 128],
                                            rows[i][:, d * 128:(d + 1) * 128],
                                            ident[:])
                    nc.vector.tensor_copy(
                        zT[:, :, i * 128:(i + 1) * 128],
                        ztp[:].rearrange("p (c n) -> p c n", c=4))
                a1 = sb.tile([128, 4, 512], BF16, tag="a1", bufs=2)
                for mc in range(4):
                    mm = p_mm.tile([128, 512], F32, tag="mm")
                    for kc in range(4):
                        nc.tensor.matmul(mm[:, :W],
                                         w1_s[:, kc, mc * 128:(mc + 1) * 128],
                                         zT[:, kc, :W],
                                         start=(kc == 0), stop=(kc == 3))
                    nc.scalar.activation(a1[:, mc, :W], mm[:, :W], act1,
                                         bias=b1_s[:, mc:mc + 1])
                for i in range(G):
                    z2 = p_z2.tile([128, 512], F32, tag="z2")
                    for kc in range(4):
                        nc.tensor.matmul(z2[:],
                                         a1[:, kc, i * 128:(i + 1) * 128],
                                         w2_s[:, kc, :],
                                         start=(kc == 0), stop=(kc == 3))
                    t = sb.tile([128, 512], BF16, tag="tb", bufs=2)
                    nc.vector.tensor_add(t[:], z2[:], b2r_s[:])
                    evac(i, t)

            # ================= PHASE 1: bond table =================
            bond_rows = []
            rsum_b = sb.tile([128, 4], F32, tag="rsum")
            ssq_b = sb.tile([128, 4], F32, tag="ssq")
            for t in range(4):
                acc = p_acc.tile([128, 512], F32, tag="acc")
                nc.tensor.matmul(acc[:], ohb_s[:, t * 128:(t + 1) * 128],
                                 bond_emb_s[:], start=True, stop=True)
                rows = sb.tile([128, 512], BF16, tag="r", bufs=5)
                nc.scalar.activation(rows[:], acc[:], AF.Identity,
                                     accum_out=rsum_b[:, t:t + 1])
                sq = sb.tile([128, 512], BF16, tag="sq")
                nc.scalar.activation(sq[:], rows[:], AF.Square,
                                     accum_out=ssq_b[:, t:t + 1])
                bond_rows.append(rows)

            rstd, nmrs = ln_stats(rsum_b, ssq_b, 4)
            bln_rows = []
            for i in range(4):
                xn = sb.tile([128, 512], BF16, tag="xn")
                nc.scalar.activation(xn[:], bond_rows[i][:], AF.Identity,
                                     scale=rstd[:, i:i + 1],
                                     bias=nmrs[:, i:i + 1])
                y = sb.tile([128, 512], BF16, tag="y")
                nc.vector.tensor_mul(y[:], xn[:], bln_g_s[:])
                lnr = sb.tile([128, 512], BF16, tag="z", bufs=8,
                              name=f"blnr{i}")
                nc.vector.tensor_add(lnr[:], y[:], bln_b_s[:])
                bln_rows.append(lnr)

            def bond_evac(i, tb):
                nc.sync.dma_start(e_table[i * 128:(i + 1) * 128, :], tb[:])

            mlp(bln_rows, 4, bw1_s, bb1_s, bw2_s, bb2r_s, AF.Gelu, bond_evac)

            # ================= PHASE 2: materialize e_edges =================
            for g in gmeta:
                ntg = g["TLg"] + g["THg"]
                gts0 = g["gts0"]
                et = sb.tile([128, NTG_MAX, 512], BF16, tag="hs", bufs=2)
                nc.gpsimd.dma_gather(
                    et[:, :ntg, :], e_table[:],
                    cmb_s[:, gts0 * 8:(gts0 + ntg) * 8],
                    ntg * 128, ntg * 128, 512, single_packet=False)
                nc.sync.dma_start(e_edges[:, gts0:gts0 + ntg, :],
                                  et[:, :ntg, :])

            # ================= PHASE 3: atom encoder =================
            for gi, grp in enumerate(GROUPS):
                G = len(grp)
                rsum = sb.tile([128, 4], F32, tag="rsum")
                ssq = sb.tile([128, 4], F32, tag="ssq")
                rows_f = []
                for i, t in enumerate(grp):
                    oht = sb.tile([128, 9, 128], BF16, tag="oha", bufs=2)
                    nc.sync.dma_start(oht[:], oha_d[:, t, :, :])
                    acc = p_acc.tile([128, 512], F32, tag="acc")
                    for f in range(9):
                        nc.tensor.matmul(acc[:], oht[:, f, :],
                                         atom_emb_s[:, f, :],
                                         start=(f == 0), stop=(f == 8))
                    rows = sb.tile([128, 512], BF16, tag="r", bufs=5)
                    nc.scalar.activation(rows[:], acc[:], AF.Identity,
                                         accum_out=rsum[:, i:i + 1])
                    sq = sb.tile([128, 512], BF16, tag="sq")
                    nc.scalar.activation(sq[:], rows[:], AF.Square,
                                         accum_out=ssq[:, i:i + 1])
                    rows_f.append(rows)
                rstd, nmrs = ln_stats(rsum, ssq, G)
                lnr_l = []
                for i in range(G):
                    xn = sb.tile([128, 512], BF16, tag="xn")
                    nc.scalar.activation(xn[:], rows_f[i][:], AF.Identity,
                                         scale=rstd[:, i:i + 1],
                                         bias=nmrs[:, i:i + 1])
                    y = sb.tile([128, 512], BF16, tag="y")
                    nc.vector.tensor_mul(y[:], xn[:], aln_g_s[:])
                    lnr = sb.tile([128, 512], BF16, tag="z", bufs=8,
                                  name=f"alnr{gi}_{i}")
                    nc.vector.tensor_add(lnr[:], y[:], aln_b_s[:])
                    lnr_l.append(lnr)

                def atom_evac(i, tb, grp=grp):
                    t = grp[i]
                    nc.sync.dma_start(
                        shard[0][t * 128:(t + 1) * 128, :], tb[:])

                mlp(lnr_l, G, aw1_s, ab1_s, aw2_s, ab2r_s, AF.Gelu, atom_evac)

            nc.gpsimd.collective_compute(
                "AllGather", ALU.bypass, replica_groups=RG,
                ins=[shard[0][:]], outs=[h_tab[0][:]])

            # ================= PHASE 4: conv layers =================
            for l in range(L):
                tab = h_tab[l % 2]
                own = shard[l % 2]
                shd = shard[(l + 1) % 2]
                cw1_s = cwp.tile([128, 4, 512], BF16, tag="cw1")
                nc.sync.dma_start(cw1_s[:], cw1[l])
                cw2_s = cwp.tile([128, 4, 512], BF16, tag="cw2")
                nc.sync.dma_start(cw2_s[:], cw2[l])
                cb1_s = cwp.tile([128, 4], F32, tag="cb1")
                nc.sync.dma_start(cb1_s[:], cb1[l])
                cb2r_s = cwp.tile([128, 512], BF16, tag="cb2r")
                nc.sync.dma_start(cb2r_s[:], cb2r[l])
                cg_s = cwp.tile([128, 512], BF16, tag="cg")
                nc.sync.dma_start(cg_s[:], cln[l, 0])
                cbe_s = cwp.tile([128, 512], BF16, tag="cbe")
                nc.sync.dma_start(cbe_s[:], cln[l, 1])

                for g in gmeta:
                    grp = g["grp"]
                    G = len(grp)
                    TLg, THg = g["TLg"], g["THg"]
                    ntg = TLg + THg
                    gts0 = g["gts0"]

                    hs = sb.tile([128, NTG_MAX, 512], BF16, tag="hs", bufs=2)
                    if TLg:
                        c0 = g["lts0"] * 8
                        nc.gpsimd.dma_gather(
                            hs[:, :TLg, :], tab[0:LO_N, :],
                            idx_lo_s[:, c0:c0 + TLg * 8],
                            TLg * 128, TLg * 128, 512, single_packet=False)
                    if THg:
                        c0 = g["hts0"] * 8
                        nc.gpsimd.dma_gather(
                            hs[:, TLg:ntg, :], tab[LO_N:NTOT, :],
                            idx_hi_s[:, c0:c0 + THg * 8],
                            THg * 128, THg * 128, 512, single_packet=False)
                    ee = sb.tile([128, NTG_MAX, 512], BF16, tag="ee",
                                 bufs=1)
                    nc.sync.dma_start(ee[:, :ntg, :],
                                      e_edges[:, gts0:gts0 + ntg, :])
                    nc.vector.tensor_add(hs[:, :ntg, :], hs[:, :ntg, :],
                                         ee[:, :ntg, :])
                    nc.vector.tensor_scalar_max(hs[:, :ntg, :],
                                                hs[:, :ntg, :], 0.0)
                    oh = sb.tile([128, NTG_MAX, 128], BF16, tag="oh", bufs=2)
                    for t in range(ntg):
                        nc.vector.tensor_scalar(
                            oh[:, t, :], iota_s[:],
                            m_bf_s[:, gts0 + t:gts0 + t + 1], None,
                            op0=ALU.is_equal)

                    hin_l = []
                    z_rows = []
                    for si, s in enumerate(grp):
                        tiles = (
                            [g["lo_off"][s] + j for j in range(int(Tl[s]))] +
                            [TLg + g["hi_off"][s] + j for j in range(int(Th[s]))])
                        agg = p_acc.tile([128, 512], F32, tag="acc")
                        for j, t in enumerate(tiles):
                            nc.tensor.matmul(agg[:], oh[:, t, :], hs[:, t, :],
                                             start=(j == 0),
                                             stop=(j == len(tiles) - 1))
                        hin = sb.tile([128, 512], BF16, tag="hin", bufs=6)
                        nc.sync.dma_start(hin[:],
                                          own[s * 128:(s + 1) * 128, :])
                        z = sb.tile([128, 512], BF16, tag="z", bufs=8)
                        nc.vector.tensor_add(z[:], agg[:], hin[:])
                        hin_l.append(hin)
                        z_rows.append(z)

                    rsum = sb.tile([128, 4], F32, tag="rsum")
                    ssq = sb.tile([128, 4], F32, tag="ssq")
                    r_l = []

                    def conv_evac(i, tb, hin_l=hin_l, rsum=rsum, ssq=ssq,
                                  r_l=r_l):
                        g2 = sb.tile([128, 512], BF16, tag="g2", bufs=2)
                        nc.scalar.activation(g2[:], tb[:], AF.Gelu)
                        r = sb.tile([128, 512], BF16, tag="r", bufs=5)
                        nc.vector.scalar_tensor_tensor(
                            r[:], g2[:], 0.0, hin_l[i][:],
                            op0=ALU.bypass, op1=ALU.add,
                            accum_out=rsum[:, i:i + 1])
                        sq = sb.tile([128, 512], BF16, tag="sq")
                        nc.scalar.activation(sq[:], r[:], AF.Square,
                                             accum_out=ssq[:, i:i + 1])
                        r_l.append(r)

                    mlp(z_rows, G, cw1_s, cb1_s, cw2_s, cb2r_s, AF.Relu,
                        conv_evac)

                    rstd, nmrs = ln_stats(rsum, ssq, G)
                    for i, s in enumerate(grp):
                        xn = sb.tile([128, 512], BF16, tag="xn")
                        nc.scalar.activation(xn[:], r_l[i][:], AF.Identity,
                                             scale=rstd[:, i:i + 1],
                                             bias=nmrs[:, i:i + 1])
                        y = sb.tile([128, 512], BF16, tag="y")
                        nc.vector.tensor_mul(y[:], xn[:], cg_s[:])
                        rs = slice(s * 128, (s + 1) * 128)
                        if l == L - 1:
                            hf = sb.tile([128, 512], F32, tag="hf")
                            nc.vector.tensor_add(hf[:], y[:], cbe_s[:])
                            nc.sync.dma_start(out_h[rs, :], hf[:])
                        else:
                            hn = sb.tile([128, 512], BF16, tag="hn")
                            nc.vector.tensor_add(hn[:], y[:], cbe_s[:])
                            nc.sync.dma_start(shd[rs, :], hn[:])
                if l < L - 1:
                    nc.gpsimd.collective_compute(
                        "AllGather", ALU.bypass, replica_groups=RG,
                        ins=[shd[:]], outs=[h_tab[(l + 1) % 2][:]])

    nc.compile()
    return nc


def kernel(x, edge_attr, edge_index,
           atom_emb, atom_ln_g, atom_ln_b, atom_w1, atom_b1, atom_w2, atom_b2,
           bond_emb, bond_ln_g, bond_ln_b, bond_w1, bond_b1, bond_w2, bond_b2,
           conv_w1, conv_b1, conv_w2, conv_b2, ln_g, ln_b):
    prep = _host_prep(x, edge_attr, edge_index)

    if prep["key"] not in _cache:
        _cache[prep["key"]] = build_program(prep)
    nc = _cache[prep["key"]]

    shared = dict(
        iota=prep["iota"], ohb=prep["ohb"],
        atom_emb=np.ascontiguousarray(
            np.asarray(atom_emb, np.float32).transpose(1, 0, 2)).astype(bf),
        bond_emb=np.asarray(bond_emb, np.float32).reshape(24, 512).astype(bf),
        aw1=_w_sb_layout(atom_w1), aw2=_w_sb_layout(atom_w2),
        bw1=_w_sb_layout(bond_w1), bw2=_w_sb_layout(bond_w2),
        cw1=np.stack([_w_sb_layout(conv_w1[l]) for l in range(L)]),
        cw2=np.stack([_w_sb_layout(conv_w2[l]) for l in range(L)]),
        ab1=_b_layout(atom_b1), bb1=_b_layout(bond_b1),
        cb1=np.stack([_b_layout(conv_b1[l]) for l in range(L)]),
        ab2r=_repl_bf(atom_b2), bb2r=_repl_bf(bond_b2),
        cb2r=np.stack([_repl_bf(conv_b2[l]) for l in range(L)]),
        aln=np.stack([_repl_bf(atom_ln_g), _repl_bf(atom_ln_b)]),
        bln=np.stack([_repl_bf(bond_ln_g), _repl_bf(bond_ln_b)]),
        cln=np.stack([np.stack([_repl_bf(ln_g[l]), _repl_bf(ln_b[l])])
                      for l in range(L)]),
    )
    in_maps = []
    for k in range(NCORES):
        m = dict(shared)
        m["idx_lo"] = prep["idx_lo"][k]
        m["idx_hi"] = prep["idx_hi"][k]
        m["cmb"] = prep["cmb"][k]
        m["m_bf"] = prep["m_bf"][k]
        m["oha"] = prep["oha"][k]
        in_maps.append(m)

    res = run_bass_kernel_spmd(nc, in_maps, list(range(NCORES)))
    kernel._last_results = res
    out = np.empty((N, D), np.float32)
    for k in range(NCORES):
        out[k * NPC:(k + 1) * NPC] = np.asarray(
            res.results[k]["out_h"], np.float32)[:NPC]
    return out
